# revision 10
# baseline (speedup 1.0000x reference)
"""Trainium2 Bass kernel for nn_Attention_70248485093458 (sliding-window attention).

Self-contained: hardcodes shapes. 8 NeuronCores, 2 query heads per core
(16 query heads, 1 shared KV head). kernel() takes full inputs, shards by
head internally (per the sharding hint), returns the full output.

Device dataflow (per core, all transposed "sT-land"):
  xT = transpose(x)                       (PE transposes)
  qT/kT = W @ xT  -> rope -> rms-scale folds -> f32r
  rms stats: ACT square of projection psums + ones-matmul partition
    reduction into a flat [1,T] layout; rs_q broadcast via gpsimd
  scoresT[tk,tq] = kT^T qT  (banded: per tk-tile, 640-wide tq span)
  attnT = exp(scale * scoresT)  (no max-subtraction; |s|<=16 provably)
  masks via gpsimd affine_select; denominators via ones-row matmuls
  att_outT[dh,tq] = sum_tk v[tk,dh]^T attnT   (512-wide tq quads)
  o[t,dm] = att_outT^T woT, normalized by 1/denominator per t (DVE)
"""

import numpy as np

import concourse.bass as bass
import concourse.mybir as mybir
import concourse.tile as tile
from concourse import bacc
from concourse.bass import ts, ds
from concourse.masks import make_identity

T = 2048
DM = 256
DH = 256
NQ = 16
WIN = 512
NCORES = 8
HPC = NQ // NCORES
P = 128
NT = T // P
EPS = 1.1920929e-07
ROPE_BASE = 10000.0
SCALE = 1.0 / 16.0

F32 = mybir.dt.float32
F32R = mybir.dt.float32r
MUL = mybir.AluOpType.mult
ADD = mybir.AluOpType.add
GE = mybir.AluOpType.is_ge
EXP = mybir.ActivationFunctionType.Exp
SQRT = mybir.ActivationFunctionType.Sqrt
COPY = mybir.ActivationFunctionType.Copy
SQUARE = mybir.ActivationFunctionType.Square


def _band(j):
    return min(WIN + P, T - P * j)


def build_program(apply_wprod: bool):
    nc = bacc.Bacc(None, target_bir_lowering=False)
    with tile.TileContext(nc) as tc, \
         tc.tile_pool(name="dram", bufs=1, space="DRAM") as dram, \
         tc.tile_pool(name="pp", bufs=1) as pp:
        x_d = dram.tile([T, DM], F32, kind="ExternalInput", name="x", uniquify=False)
        wqT_d = dram.tile([DM, HPC * DH], F32, kind="ExternalInput", name="wqT", uniquify=False)
        wkT_d = dram.tile([DM, DH], F32, kind="ExternalInput", name="wkT", uniquify=False)
        wvT_d = dram.tile([DM, DH], F32, kind="ExternalInput", name="wvT", uniquify=False)
        woT_d = dram.tile([HPC * DH, DM], F32, kind="ExternalInput", name="woT", uniquify=False)
        cos_d = dram.tile([P, T], F32, kind="ExternalInput", name="cosx", uniquify=False)
        sin_d = dram.tile([P, T], F32, kind="ExternalInput", name="sinx", uniquify=False)
        wprod_d = None
        if apply_wprod:
            wprod_d = dram.tile([P, 2], F32, kind="ExternalInput", name="wprod", uniquify=False)
        o_d = dram.tile([T, DM], F32, kind="ExternalOutput", name="o", uniquify=False)
        rsk_d = dram.tile([1, T], F32, kind="Internal", name="rskflat")
        denflat_d = [
            dram.tile([1, T], F32, kind="Internal", name=f"denflat{h}") for h in range(HPC)
        ]

        # ---- persistent SBUF ----
        xT = pp.tile([P, 2, T], F32R)
        qTr = pp.tile([P, 2 * HPC, T], F32R)
        kTr = pp.tile([P, 2, T], F32R)
        v_sb = pp.tile([P, NT, DH], F32R)
        woT_sb = pp.tile([P, 2 * HPC, DM], F32R)
        o_sb = pp.tile([P, NT, DM], F32)
        rs_k = pp.tile([P, NT], F32)          # natural layout, includes 1/16
        rden = pp.tile([P, HPC, NT], F32)
        ones_col = pp.tile([P, 1], F32R)
        ident = pp.tile([P, P], F32)
        wprod_sb = pp.tile([P, 2], F32) if apply_wprod else None

        nc.gpsimd.dma_start(out=woT_sb[:], in_=woT_d.rearrange("(c p) n -> p c n", p=P))
        make_identity(nc, ident[:])
        ones_f32 = pp.tile([P, 1], F32)
        nc.vector.memset(ones_f32[:], 1.0)
        nc.vector.tensor_copy(ones_col[:], ones_f32[:])

        wpa_cm = tc.tile_pool(name="wp", bufs=1)
        psa_cm = tc.tile_pool(name="psa", bufs=1, space="PSUM")
        wp = wpa_cm.__enter__()
        psa = psa_cm.__enter__()
        cos_sb = wp.tile([P, T], F32)
        sin_sb = wp.tile([P, T], F32)
        negsin = wp.tile([P, T], F32)
        wqT_sb = wp.tile([P, 2, HPC * DH], F32R)
        wkT_sb = wp.tile([P, 2, DH], F32R)
        wvT_sb = wp.tile([P, 2, DH], F32R)
        nc.gpsimd.dma_start(out=wqT_sb[:], in_=wqT_d.rearrange("(c p) n -> p c n", p=P))
        nc.gpsimd.dma_start(out=wkT_sb[:], in_=wkT_d.rearrange("(c p) n -> p c n", p=P))
        nc.gpsimd.dma_start(out=wvT_sb[:], in_=wvT_d.rearrange("(c p) n -> p c n", p=P))
        nc.sync.dma_start(out=cos_sb[:], in_=cos_d[:])
        nc.sync.dma_start(out=sin_sb[:], in_=sin_d[:])
        nc.vector.tensor_scalar(
            out=negsin[:], in0=sin_sb[:], scalar1=-1.0, scalar2=0.0, op0=MUL, op1=ADD
        )
        if apply_wprod:
            nc.sync.dma_start(out=wprod_sb[:], in_=wprod_d[:])

        # ---- transpose x ----
        for n in range(NT):
            x_t = wp.tile([P, DM], F32, tag="x_t", bufs=2, name=f"x_t{n}")
            nc.sync.dma_start(out=x_t[:], in_=x_d[ts(n, P), :])
            tx_ps = psa.tile([P, DM], F32, tag="tx", bufs=2, name=f"txps{n}")
            for c in range(2):
                nc.tensor.transpose(tx_ps[:, ts(c, P)], x_t[:, ts(c, P)], ident[:])
            nc.scalar.activation(
                xT[:, :, ts(n, P)], tx_ps[:].rearrange("p (c q) -> p c q", c=2), COPY
            )

        # ---- v projection ----
        for n in range(NT):
            v_ps = psa.tile([P, DH], F32, tag="proj", bufs=2, name=f"vps{n}")
            for c in range(2):
                nc.tensor.matmul(
                    v_ps[:], xT[:, c, ts(n, P)], wvT_sb[:, c, :],
                    start=(c == 0), stop=(c == 1),
                )
            nc.scalar.activation(v_sb[:, n, :], v_ps[:], COPY)

        # ---- k projection + squares + rope ----
        kT_sb = wp.tile([P, 2, T], F32)
        ssqk_flat = wp.tile([1, T], F32, name="ssqk_flat")
        for nb in range(4):
            sqk = [None, None]
            for m in range(2):
                k_ps = psa.tile([P, 512], F32, tag="proj2", bufs=2, name=f"kps{nb}_{m}")
                for c in range(2):
                    nc.tensor.matmul(
                        k_ps[:], wkT_sb[:, c, ts(m, P)], xT[:, c, ts(nb, 512)],
                        start=(c == 0), stop=(c == 1),
                    )
                nc.scalar.activation(kT_sb[:, m, ts(nb, 512)], k_ps[:], COPY)
                sq = wp.tile([P, 512], F32R, tag="sq", bufs=2, name=f"sqk{nb}_{m}")
                nc.scalar.activation(sq[:], k_ps[:], SQUARE)
                sqk[m] = sq
            ss_ps = psa.tile([1, 512], F32, tag="ssq", bufs=2, name=f"sskps{nb}")
            for m in range(2):
                nc.tensor.matmul(
                    ss_ps[:], ones_col[:], sqk[m][:], start=(m == 0), stop=(m == 1)
                )
            nc.vector.tensor_copy(ssqk_flat[:, ts(nb, 512)], ss_ps[:])
        # rs_k on the flat layout: 1/sqrt(ssq/DH+eps) * 1/16
        nc.vector.tensor_scalar(
            out=ssqk_flat[:], in0=ssqk_flat[:], scalar1=1.0 / DH, scalar2=EPS,
            op0=MUL, op1=ADD,
        )
        nc.scalar.activation(ssqk_flat[:], ssqk_flat[:], SQRT)
        nc.vector.reciprocal(ssqk_flat[:], ssqk_flat[:])
        nc.vector.tensor_scalar(
            out=ssqk_flat[:], in0=ssqk_flat[:], scalar1=SCALE, scalar2=0.0, op0=MUL, op1=ADD
        )
        # flat -> natural [P, NT] via DRAM bounce + PE transpose
        nc.sync.dma_start(out=rsk_d[:], in_=ssqk_flat[:])
        rsk_rows = wp.tile([NT, P], F32, name="rsk_rows")
        nc.sync.dma_start(out=rsk_rows[:], in_=rsk_d.rearrange("a (n q) -> (a n) q", q=P))
        rkt_ps = psa.tile([P, NT], F32, tag="ssq", bufs=2, name="rktps")
        nc.tensor.transpose(rkt_ps[:, 0:NT], rsk_rows[:], ident[0:NT, 0:NT])
        nc.vector.tensor_copy(rs_k[:], rkt_ps[:, 0:NT])

        # k rope on gpsimd
        ke, ko = kT_sb[:, 0, :], kT_sb[:, 1, :]
        for half in range(2):
            sp = ts(half, T // 2)
            u1 = wp.tile([P, T // 2], F32, tag="krope", bufs=2, name=f"u1_{half}")
            u2 = wp.tile([P, T // 2], F32, tag="krope", bufs=2, name=f"u2_{half}")
            nc.gpsimd.tensor_mul(u1[:], ke[:, sp], cos_sb[:, sp])
            nc.gpsimd.tensor_mul(u2[:], ko[:, sp], negsin[:, sp])
            if apply_wprod:
                kre = wp.tile([P, T // 2], F32, tag="krope", bufs=2, name=f"kre{half}")
                nc.gpsimd.tensor_add(kre[:], u1[:], u2[:])
                nc.vector.tensor_scalar(
                    out=kTr[:, 0, sp], in0=kre[:], scalar1=wprod_sb[:, 0:1],
                    scalar2=0.0, op0=MUL, op1=ADD,
                )
            else:
                nc.gpsimd.tensor_add(kTr[:, 0, sp], u1[:], u2[:])
            u3 = wp.tile([P, T // 2], F32, tag="krope", bufs=2, name=f"u3_{half}")
            u4 = wp.tile([P, T // 2], F32, tag="krope", bufs=2, name=f"u4_{half}")
            nc.gpsimd.tensor_mul(u3[:], ke[:, sp], sin_sb[:, sp])
            nc.gpsimd.tensor_mul(u4[:], ko[:, sp], cos_sb[:, sp])
            if apply_wprod:
                kro = wp.tile([P, T // 2], F32, tag="krope", bufs=2, name=f"kro{half}")
                nc.gpsimd.tensor_add(kro[:], u3[:], u4[:])
                nc.vector.tensor_scalar(
                    out=kTr[:, 1, sp], in0=kro[:], scalar1=wprod_sb[:, 1:2],
                    scalar2=0.0, op0=MUL, op1=ADD,
                )
            else:
                nc.gpsimd.tensor_add(kTr[:, 1, sp], u3[:], u4[:])

        # ---- q projection + rope (per head) + rs_q ----
        rsq_flat = [
            wp.tile([1, T], F32, tag="rsq_flat", bufs=1, name=f"rsqf{h}") for h in range(HPC)
        ]
        rsq_bc = [
            wp.tile([P, T], F32, tag="rsq_bc", bufs=1, name=f"rqb{h}") for h in range(HPC)
        ]
        for h in range(HPC):
            for nb in range(4):
                sp = ts(nb, 512)
                qe_ps = psa.tile([P, 512], F32, tag="proj2", bufs=2, name=f"qeps{h}_{nb}")
                qo_ps = psa.tile([P, 512], F32, tag="proj", bufs=2, name=f"qops{h}_{nb}")
                for c in range(2):
                    nc.tensor.matmul(
                        qe_ps[:], wqT_sb[:, c, ts(2 * h, P)], xT[:, c, sp],
                        start=(c == 0), stop=(c == 1),
                    )
                for c in range(2):
                    nc.tensor.matmul(
                        qo_ps[:], wqT_sb[:, c, ts(2 * h + 1, P)], xT[:, c, sp],
                        start=(c == 0), stop=(c == 1),
                    )
                # squares -> partition-reduce (flat layout)
                sqe = wp.tile([P, 512], F32R, tag="sq", bufs=2, name=f"sqe{h}_{nb}")
                sqo = wp.tile([P, 512], F32R, tag="sq", bufs=2, name=f"sqo{h}_{nb}")
                nc.scalar.activation(sqe[:], qe_ps[:], SQUARE)
                nc.scalar.activation(sqo[:], qo_ps[:], SQUARE)
                ss_ps = psa.tile([1, 512], F32, tag="ssq", bufs=2, name=f"ssq{h}_{nb}")
                nc.tensor.matmul(ss_ps[:], ones_col[:], sqe[:], start=True, stop=False)
                nc.tensor.matmul(ss_ps[:], ones_col[:], sqo[:], start=False, stop=True)
                nc.vector.tensor_copy(rsq_flat[h][:, sp], ss_ps[:])
                # rope multiplies (DVE, psum-sourced)
                t1 = wp.tile([P, 512], F32, tag="qrope", bufs=2, name=f"t1_{h}{nb}")
                t2 = wp.tile([P, 512], F32, tag="qrope", bufs=2, name=f"t2_{h}{nb}")
                nc.vector.tensor_mul(t1[:], qe_ps[:], cos_sb[:, sp])
                nc.vector.tensor_mul(t2[:], qo_ps[:], sin_sb[:, sp])
                nc.vector.tensor_sub(qTr[:, 2 * h, sp], t1[:], t2[:])
                t3 = wp.tile([P, 512], F32, tag="qrope", bufs=2, name=f"t3_{h}{nb}")
                t4 = wp.tile([P, 512], F32, tag="qrope", bufs=2, name=f"t4_{h}{nb}")
                nc.vector.tensor_mul(t3[:], qe_ps[:], sin_sb[:, sp])
                nc.vector.tensor_mul(t4[:], qo_ps[:], cos_sb[:, sp])
                nc.vector.tensor_add(qTr[:, 2 * h + 1, sp], t3[:], t4[:])
            # rs_q on flat layout, then partition-broadcast
            nc.vector.tensor_scalar(
                out=rsq_flat[h][:], in0=rsq_flat[h][:], scalar1=1.0 / DH, scalar2=EPS,
                op0=MUL, op1=ADD,
            )
            nc.scalar.activation(rsq_flat[h][:], rsq_flat[h][:], SQRT)
            nc.vector.reciprocal(rsq_flat[h][:], rsq_flat[h][:])
            nc.gpsimd.partition_broadcast(rsq_bc[h][:], rsq_flat[h][:], channels=P)
            # folds on DVE, in place
            for nb in range(4):
                sp = ts(nb, 512)
                for par in range(2):
                    nc.vector.tensor_mul(
                        qTr[:, 2 * h + par, sp], qTr[:, 2 * h + par, sp], rsq_bc[h][:, sp]
                    )

        psa_cm.__exit__(None, None, None)
        wpa_cm.__exit__(None, None, None)

        # =================== attention ===================
        with (
            tc.tile_pool(name="wpb", bufs=1) as wp2,
            tc.tile_pool(name="ps_b", bufs=1, space="PSUM") as psb,
        ):
            for h in range(HPC):
                den_sb = wp2.tile([1, T], F32, tag="den_sb", bufs=2, name=f"den{h}")
                attoT = wp2.tile([P, 2, T], F32R, tag="attoT", bufs=2, name=f"aoT{h}")
                attnT = {}

                for j in range(NT):
                    w = _band(j)
                    s_ps = psb.tile([P, 1024], F32, tag="qk", bufs=2, name=f"sps{h}_{j}")
                    if w <= 512:
                        for c in range(2):
                            nc.tensor.matmul(
                                s_ps[:, 0:w], kTr[:, c, ts(j, P)],
                                qTr[:, 2 * h + c, ds(P * j, w)],
                                start=(c == 0), stop=(c == 1),
                            )
                    else:
                        for c in range(2):
                            nc.tensor.matmul(
                                s_ps[:, 0:320], kTr[:, c, ts(j, P)],
                                qTr[:, 2 * h + c, ds(P * j, 320)],
                                start=(c == 0), stop=(c == 1),
                            )
                        for c in range(2):
                            nc.tensor.matmul(
                                s_ps[:, 512:832], kTr[:, c, ts(j, P)],
                                qTr[:, 2 * h + c, ds(P * j + 320, 320)],
                                start=(c == 0), stop=(c == 1),
                            )
                    at = wp2.tile([P, 640], F32R, tag="attnT", bufs=10, name=f"at{h}_{j}")
                    attnT[j] = at
                    if w <= 512:
                        nc.scalar.activation(
                            at[:, 0:w], s_ps[:, 0:w], EXP, scale=rs_k[:, j : j + 1]
                        )
                    else:
                        nc.scalar.activation(
                            at[:].rearrange("p (b q) -> p b q", b=2),
                            s_ps[:].rearrange("p (b q) -> p b q", q=512)[:, :, 0:320],
                            EXP, scale=rs_k[:, j : j + 1],
                        )
                    nc.gpsimd.affine_select(
                        out=at[:, 0:P], in_=at[:, 0:P], compare_op=GE, fill=0.0,
                        base=0, pattern=[[1, P]], channel_multiplier=-1,
                    )
                    if w == WIN + P:
                        nc.gpsimd.affine_select(
                            out=at[:, WIN : WIN + P], in_=at[:, WIN : WIN + P],
                            compare_op=GE, fill=0.0,
                            base=0, pattern=[[-1, P]], channel_multiplier=1,
                        )

                    if j % 4 == 3:
                        _emit_quad(nc, psb, h, j // 4, attnT, v_sb, ones_col, den_sb, attoT)

                # denominators -> natural layout; reciprocal
                nc.sync.dma_start(out=denflat_d[h][:], in_=den_sb[:])
                den_rows = wp2.tile([NT, P], F32, tag="den_rows", bufs=2, name=f"denr{h}")
                nc.sync.dma_start(
                    out=den_rows[:], in_=denflat_d[h].rearrange("a (n q) -> (a n) q", q=P)
                )
                d_ps = psb.tile([P, NT], F32, tag="den", bufs=2, name=f"dtp{h}")
                nc.tensor.transpose(d_ps[:, 0:NT], den_rows[:], ident[0:NT, 0:NT])
                nc.vector.tensor_copy(rden[:, h, :], d_ps[:, 0:NT])
                nc.vector.reciprocal(rden[:, h, :], rden[:, h, :])

                # output projection + normalize/accumulate
                for n in range(NT):
                    o_ps = psb.tile([P, DM], F32, tag="pv", bufs=2, name=f"ops{h}_{n}")
                    for c in range(2):
                        nc.tensor.matmul(
                            o_ps[:], attoT[:, c, ts(n, P)], woT_sb[:, 2 * h + c, :],
                            start=(c == 0), stop=(c == 1),
                        )
                    if h == 0:
                        nc.vector.tensor_scalar(
                            out=o_sb[:, n, :], in0=o_ps[:],
                            scalar1=rden[:, 0, n : n + 1], scalar2=0.0, op0=MUL, op1=ADD,
                        )
                    else:
                        nc.vector.scalar_tensor_tensor(
                            out=o_sb[:, n, :], in0=o_ps[:], scalar=rden[:, 1, n : n + 1],
                            in1=o_sb[:, n, :], op0=MUL, op1=ADD,
                        )

            nc.sync.dma_start(out=o_d.rearrange("(n p) d -> p n d", p=P), in_=o_sb[:])
    nc.compile()
    return nc


def _emit_quad(nc, psb, h, q, attnT, v_sb, ones_col, den_sb, attoT):
    """PV + denominator matmuls for tq-quad q (columns [512q, 512q+512)) of head h."""
    Q = 512
    js = list(range(max(0, 4 * q - 4), min(NT, 4 * q + 4)))
    # a full-width tk-tile first so psum has_written stays uniform per matmul
    js.remove(4 * q)
    js.insert(0, 4 * q)
    a_ps = [
        psb.tile([P, Q], F32, tag="pv", bufs=2, name=f"aq{h}_{q}_0"),
        psb.tile([P, Q], F32, tag="pv", bufs=2, name=f"aq{h}_{q}_1"),
    ]
    d_ps = psb.tile([1, Q], F32, tag="den", bufs=2, name=f"dq{h}_{q}")
    for ji, j in enumerate(js):
        w = _band(j)
        lo = max(0, Q * q - P * j)
        hi = min(w, Q * q + Q - P * j)
        assert hi > lo
        po = P * j + lo - Q * q
        start = ji == 0
        stop = ji == len(js) - 1
        src = attnT[j][:, lo:hi]
        for c in range(2):
            nc.tensor.matmul(
                a_ps[c][:, po : po + hi - lo], v_sb[:, j, ts(c, P)], src,
                start=start, stop=stop,
            )
        nc.tensor.matmul(
            d_ps[:, po : po + hi - lo], ones_col[:], src, start=start, stop=stop
        )
    for c in range(2):
        nc.scalar.activation(attoT[:, c, ts(q, Q)], a_ps[c][:], COPY)
    nc.vector.tensor_copy(den_sb[:, ts(q, Q)], d_ps[:])


# ======================= host side =======================

_PROGRAMS = {}


def _get_program(apply_wprod: bool):
    key = bool(apply_wprod)
    if key not in _PROGRAMS:
        _PROGRAMS[key] = build_program(key)
    return _PROGRAMS[key]


_DEINT = np.concatenate([np.arange(0, DH, 2), np.arange(1, DH, 2)])


def _rope_tables():
    freqs = (ROPE_BASE ** (-2.0 * np.arange(DH // 2, dtype=np.float32) / DH)).astype(np.float32)
    theta = np.arange(T, dtype=np.float32)[None, :] * freqs[:, None]
    return np.cos(theta).astype(np.float32), np.sin(theta).astype(np.float32)


def _prep_inputs(x, wq, wkv, wo, q_norm_w, k_norm_w):
    x2 = np.ascontiguousarray(np.asarray(x, dtype=np.float32).reshape(T, DM))
    wq = np.asarray(wq, dtype=np.float32)
    wkv = np.asarray(wkv, dtype=np.float32)
    wo = np.asarray(wo, dtype=np.float32)
    wk = wkv[:DH]
    wv = wkv[DH:]
    cos, sin = _rope_tables()

    wprod = (np.asarray(q_norm_w, np.float32) * np.asarray(k_norm_w, np.float32))
    apply_wprod = not np.allclose(wprod, 1.0)
    wprod_de = wprod[_DEINT]
    wprod_cols = np.ascontiguousarray(
        np.stack([wprod_de[: DH // 2], wprod_de[DH // 2 :]], axis=1)
    )

    wkT = np.ascontiguousarray(wk[_DEINT].T)     # [DM, DH]
    wvT = np.ascontiguousarray(wv.T)             # [DM, DH]

    in_maps = []
    for c in range(NCORES):
        heads = [HPC * c + i for i in range(HPC)]
        wq_rows = np.concatenate([wq[DH * h : DH * (h + 1)][_DEINT] for h in heads], axis=0)
        wqT = np.ascontiguousarray(wq_rows.T)    # [DM, HPC*DH]
        woT = np.ascontiguousarray(wo[:, DH * heads[0] : DH * (heads[-1] + 1)].T)
        m = {
            "x": x2, "wqT": wqT, "wkT": wkT, "wvT": wvT, "woT": woT,
            "cosx": cos, "sinx": sin,
        }
        if apply_wprod:
            m["wprod"] = wprod_cols
        in_maps.append(m)
    return in_maps, apply_wprod


def _run(inputs, trace=False, trace_kwargs=None):
    from concourse.bass_utils import run_bass_kernel_spmd

    in_maps, apply_wprod = _prep_inputs(**inputs)
    nc = _get_program(apply_wprod)
    res = run_bass_kernel_spmd(
        nc, in_maps, list(range(NCORES)), trace=trace, **(trace_kwargs or {})
    )
    out = np.zeros((T, DM), dtype=np.float32)
    for c in range(NCORES):
        out += res.results[c]["o"]
    return out.reshape(1, T, DM), res


def kernel(**inputs):
    out, _ = _run(inputs, trace=False)
    return out
